# revision 3
# baseline (speedup 1.0000x reference)
"""Euclidean distance layer on 8 Trainium2 NeuronCores.

out[b, o] = || x[b, :] - weight[:, o] ||_2
x: [512, 256] f32, weight: [256, 1024] f32 -> out: [512, 1024] f32

Strategy: tensor-parallel over output features (8 x 128 columns).
Each core computes dist^2 = -2 * (x@w_loc - 0.5*||x||^2 - 0.5*||w_loc||^2)
entirely on the PE array:
  - main matmuls: psum[m] += xT[k,m-tile].T @ w_loc[k]         (K-chunks of 128)
  - norm rows via ones-vector reduce matmuls (partition-dim reduction)
  - a K=2 augmented matmul folds both norm terms into the same PSUM banks
  - final ACT op: out = sqrt(-2 * psum)
Host work is layout-only: transpose x (so K lands on partitions), slice w,
concat output slices.
"""

from contextlib import ExitStack

import numpy as np

B = 512      # batch
K = 256      # inputSize (contraction dim)
NOUT = 1024  # outputSize
NCORES = 8
NLOC = NOUT // NCORES  # 128 output features per core
P = 128                # partitions
KT = K // P            # 2 contraction chunks
MT = B // P            # 4 batch tiles

_NC = None  # cached compiled Bass program (shared SPMD across all 8 cores)


def _build():
    import concourse.bass as bass
    import concourse.tile as tile
    from concourse import bacc, mybir

    f32 = mybir.dt.float32
    nc = bacc.Bacc(
        "TRN2", target_bir_lowering=False, debug=False, num_devices=NCORES
    )

    xt = nc.dram_tensor("xt", [K, B], f32, kind="ExternalInput")
    wl = nc.dram_tensor("wl", [K, NLOC], f32, kind="ExternalInput")
    out = nc.dram_tensor("out", [B, NLOC], f32, kind="ExternalOutput")

    with tile.TileContext(nc) as tc, ExitStack() as ctx:
        pool = ctx.enter_context(tc.tile_pool(name="sb", bufs=1))
        ppool = ctx.enter_context(tc.tile_pool(name="ps", bufs=1, space="PSUM"))

        # ---- loads ----
        xt_sb, wl_sb, xtsq, wlsq = [], [], [], []
        for k in range(KT):
            t = pool.tile([P, B], f32, tag=f"xt{k}", name=f"xt{k}")
            nc.sync.dma_start(out=t, in_=xt[k * P : (k + 1) * P, :])
            xt_sb.append(t)
            t = pool.tile([P, NLOC], f32, tag=f"wl{k}", name=f"wl{k}")
            nc.sync.dma_start(out=t, in_=wl[k * P : (k + 1) * P, :])
            wl_sb.append(t)

        # ---- elementwise squares (DVE) ----
        for k in range(KT):
            t = pool.tile([P, B], f32, tag=f"xtsq{k}", name=f"xtsq{k}")
            nc.vector.tensor_mul(t, xt_sb[k], xt_sb[k])
            xtsq.append(t)
            t = pool.tile([P, NLOC], f32, tag=f"wlsq{k}", name=f"wlsq{k}")
            nc.vector.tensor_mul(t, wl_sb[k], wl_sb[k])
            wlsq.append(t)

        # ---- constants ----
        # lhsT for the partition-dim reductions: [128, 2] of -0.5 (M=2 so the
        # w-norm lands on PSUM rows 0..1; engine writes must start at part 0)
        neg_half = pool.tile([P, 2], f32, tag="neg_half")
        nc.vector.memset(neg_half, -0.5)
        aug_l = pool.tile([2, B], f32, tag="aug_l")   # [-0.5*xsq ; ones]
        nc.vector.memset(aug_l, 1.0)
        aug_r = pool.tile([2, NLOC], f32, tag="aug_r")  # [ones ; -0.5*wsq]

        # ---- PSUM ----
        ps_xsq = ppool.tile([1, B], f32, tag="ps_xsq")
        ps_wsq = ppool.tile([2, NLOC], f32, tag="ps_wsq")
        ps_m = [ppool.tile([P, NLOC], f32, tag=f"ps{m}", name=f"ps{m}") for m in range(MT)]

        # ---- norm reductions: -0.5 * sum_k v[k]^2 along partitions ----
        for k in range(KT):
            nc.tensor.matmul(
                ps_xsq, lhsT=neg_half[:, 0:1], rhs=xtsq[k],
                start=(k == 0), stop=(k == KT - 1),
            )
        for k in range(KT):
            nc.tensor.matmul(
                ps_wsq, lhsT=neg_half, rhs=wlsq[k],
                start=(k == 0), stop=(k == KT - 1),
            )
        # aug_l row0 <- -0.5*xsq (row1 stays 1.0 from the memset above)
        nc.vector.tensor_copy(aug_l[0:1, :], ps_xsq)
        # aug_r rows <- [-0.5*wsq ; -0.5*wsq], then row0 overwritten with 1.0
        nc.vector.tensor_copy(aug_r, ps_wsq)
        nc.vector.memset(aug_r[0:1, :], 1.0)

        # ---- main matmuls: psum[m] = x @ w_loc (batch tile m) ----
        for m in range(MT):
            for k in range(KT):
                nc.tensor.matmul(
                    ps_m[m],
                    lhsT=xt_sb[k][:, bass.ts(m, P)],
                    rhs=wl_sb[k],
                    start=(k == 0), stop=False,
                )

        # ---- fold norms in, then sqrt(-2 * psum) ----
        out_sb = pool.tile([P, MT, NLOC], f32, tag="out_sb")
        for m in range(MT):
            nc.tensor.matmul(
                ps_m[m],
                lhsT=aug_l[:, bass.ts(m, P)],
                rhs=aug_r,
                start=False, stop=True,
            )
            nc.scalar.activation(
                out_sb[:, m, :], ps_m[m],
                func=mybir.ActivationFunctionType.Sqrt,
                scale=-2.0,
            )

        nc.sync.dma_start(
            out=out[:, :].rearrange("(m p) o -> p m o", p=P), in_=out_sb
        )

    nc.compile()
    return nc


def _get_nc():
    global _NC
    if _NC is None:
        _NC = _build()
    return _NC


def _make_in_maps(x: np.ndarray, weight: np.ndarray):
    xt = np.ascontiguousarray(x.T.astype(np.float32, copy=False))
    return [
        {
            "xt": xt,
            "wl": np.ascontiguousarray(weight[:, c * NLOC : (c + 1) * NLOC]),
        }
        for c in range(NCORES)
    ]


def run(x: np.ndarray, weight: np.ndarray, trace: bool = False):
    """Returns (full_output, BassKernelResults)."""
    from concourse.bass_utils import run_bass_kernel_spmd

    nc = _get_nc()
    res = run_bass_kernel_spmd(
        nc, _make_in_maps(x, weight), core_ids=list(range(NCORES)), trace=trace
    )
    full = np.concatenate(
        [res.results[c]["out"] for c in range(NCORES)], axis=1
    )
    return full, res


def kernel(x: np.ndarray, weight: np.ndarray) -> np.ndarray:
    return run(x, weight)[0]
